# revision 32
# baseline (speedup 1.0000x reference)
"""Trainium2 Bass kernel for nn_CASTransformerLayer_46943992545734.

Strategy (8 NeuronCores, no collectives):
  Pure data-parallel over the sequence axis: core c handles tokens
  [c*512, (c+1)*512) of BOTH batches, with an 8-token halo on key/values
  for the dilated sliding-window attention (max reach = 2*dil = 6).

Per-core pipeline (all layouts channel-on-partitions "cT" unless noted):
  1. Host pre-permutes weights so the reference's "faithful raw reshape"
     head scrambling becomes contiguous:  proj output channel order
     c_new = d*16 + g  (g = scrambled head, d = scrambled head-channel).
     In that order the probs multiplier is partition-periodic mod 16.
  2. xbar-transpose DMA loads X^T (bf16) tiles; PE computes q/k/v = W'X^T
     in [c_new, t] layout (bf16 matmuls, fp32 PSUM).
  3. scores: DVE forms q ⊙ k_shift per tap (shifted APs; zero-padded halo
     reproduces the reference's zero-pad semantics exactly), PE reduces
     over d with a 0/1 pattern matmul -> scores [16 g, t]; softmax DVE/ACT.
  4. probs replicated to 128 partitions with a second pattern matmul;
     DVE applies taps to v and tree-sums -> attn [c_new, t] bf16.
  5. out_proj: PE matmuls with column-permuted Wo^T -> y [t, o] fp32 PSUM;
     residual add + LayerNorm on DVE; outputs y fp32 and probs fp32.

b0/b1/bo/ln_* are applied generally (bias/mask/affine paths are emitted
only when the host sees nonzero values, which keeps the hot path lean for
this problem's all-zero biases).
"""

import sys

for _p in ("/opt/trn_rl_repo",):
    if _p not in sys.path:
        sys.path.insert(0, _p)

import numpy as np
import ml_dtypes

BS, SEQ, DM = 2, 4096, 2048
NCORES = 8
TB = SEQ // NCORES            # 512 tokens per batch per core
HALO = 8                      # >= 2*max(dil); 528 % 16 == 0 for xbar tiles
TK = TB + 2 * HALO            # 528
SPLIT = DM // 2               # 1024
KS = 5
DILS = (3, 1)
NG, DG = 16, 64               # scrambled heads per branch, channels per head
NCC = SPLIT // 128            # 8 c_new chunks per branch
LN_EPS = 1e-5
SCALE = DG ** -0.5

BF16 = ml_dtypes.bfloat16

_CACHE = {}


def _perms():
    cn = np.arange(SPLIT)
    g = cn % NG
    d = cn // NG
    h = d % 16
    r = d // 16
    perm_o = h * 64 + 4 * g + r    # proj output channel for c_new
    perm_a = g * 64 + d            # attn channel for c_new
    return perm_o, perm_a


def _build_program(has_bias, has_affine):
    import concourse.bass as bass
    import concourse.tile as tile
    from concourse import bacc, mybir

    f32 = mybir.dt.float32
    bf16 = mybir.dt.bfloat16
    Alu = mybir.AluOpType
    Act = mybir.ActivationFunctionType
    Ax = mybir.AxisListType

    nc = bacc.Bacc("TRN2", target_bir_lowering=False, debug=False)

    def din(name, shape, dt):
        return nc.dram_tensor(name, shape, dt, kind="ExternalInput").ap()

    xq = din("xq", [BS, TB, DM], bf16)
    xk = din("xk", [BS, TK, DM], bf16)
    xv = din("xv", [BS, TK, DM], bf16)
    qres = din("qres", [BS, TB, DM], f32)        # query slice + bo (fp32)
    wp = din("wp", [2, SPLIT, SPLIT], bf16)      # [br][ci][co] = W'_br^T
    wot = din("wot", [DM, DM], bf16)             # [c_new_all][o] col-permuted Wo^T
    pat_sum = din("pat_sum", [128, NG], bf16)    # d-sum pattern
    pat_rep = din("pat_rep", [NG, 128], bf16)    # g-replication pattern
    if has_bias:
        bias_in = din("bias", [128, 2, NCC], f32)       # [p][br][co] permuted b
        bias_q = din("bias_q", [128, 2, NCC], f32)      # same, pre-scaled by 0.125
        mask_in = din("maskkv", [BS, TK], f32)          # 1 in-sequence, 0 on pads
    if has_affine:
        lnw_in = din("ln_w", [DM], f32)
        lnb_in = din("ln_b", [DM], f32)

    y_out = nc.dram_tensor("y", [BS, TB, DM], f32, kind="ExternalOutput").ap()
    p_out = nc.dram_tensor("probs", [BS, 2 * NG, TB, KS], f32, kind="ExternalOutput").ap()

    import os
    dbg = bool(os.environ.get("KERNEL_DEBUG_DUMPS"))
    if dbg:
        dq = nc.dram_tensor("dbg_qp", [2, 128, NCC, BS, TB], bf16, kind="ExternalOutput").ap()
        dk = nc.dram_tensor("dbg_kp", [2, 128, NCC, BS, TK], bf16, kind="ExternalOutput").ap()
        dS = nc.dram_tensor("dbg_S", [2, BS, NG, KS, TB], f32, kind="ExternalOutput").ap()
        dA = nc.dram_tensor("dbg_attn", [2, 128, NCC, BS, TB], bf16, kind="ExternalOutput").ap()


    def bcast_free(ap_p1, *dims):
        # [p, 1] -> [p, *dims] via step-0 free broadcast
        return bass.AP(tensor=ap_p1.tensor, offset=ap_p1.offset,
                       ap=[ap_p1.ap[0]] + [[0, d] for d in dims])

    def bcast_mid(ap_pt, n):
        # [p, t] -> [p, n, t]: broadcast a new middle dim (step 0), keep t
        return bass.AP(tensor=ap_pt.tensor, offset=ap_pt.offset,
                       ap=[ap_pt.ap[0], [0, n]] + [list(d) for d in ap_pt.ap[1:]])

    def pbcast(ap_1d, parts):
        # DRAM [n] (or [a, b]) -> partition-broadcast AP [parts, ...]
        return bass.AP(tensor=ap_1d.tensor, offset=ap_1d.offset,
                       ap=[[0, parts]] + [list(d) for d in ap_1d.ap])

    with tile.TileContext(nc) as tc:
        with tc.tile_pool(name="const", bufs=1) as consts:
            psum_pat = consts.tile([128, NG], bf16)
            nc.sync.dma_start(out=psum_pat, in_=pat_sum)
            erep_pat = consts.tile([NG, 128], bf16)
            nc.sync.dma_start(out=erep_pat, in_=pat_rep)
            eps_t = consts.tile([128, 1], f32)
            nc.vector.memset(eps_t, LN_EPS)
            if has_bias:
                bias_t = consts.tile([128, 2, NCC], f32)
                nc.sync.dma_start(out=bias_t, in_=bias_in)
                biasq_t = consts.tile([128, 2, NCC], f32)
                nc.sync.dma_start(out=biasq_t, in_=bias_q)
                mask_t = consts.tile([128, BS, TK], f32)
                nc.sync.dma_start(out=mask_t, in_=pbcast(mask_in, 128))
            if has_affine:
                lnw_t = consts.tile([128, DM], f32)
                nc.sync.dma_start(out=lnw_t, in_=pbcast(lnw_in, 128))
                lnb_t = consts.tile([128, DM], f32)
                nc.sync.dma_start(out=lnb_t, in_=pbcast(lnb_in, 128))

            with tc.tile_pool(name="attn", bufs=2) as attn_pool:

                # ===== phase 1+2: projections + attention, per branch =====
                # NCC(8) XT tiles stay live across each projection pass's co
                # loop, so the shared tag needs >= NCC+1 slots or the 9th
                # allocation deadlocks against consumers emitted later.
                with tc.tile_pool(name="proj", bufs=1) as proj_pool, \
                     tc.tile_pool(name="xt", bufs=12) as xt_pool, \
                     tc.tile_pool(name="wts", bufs=1) as wpool, \
                     tc.tile_pool(name="sm", bufs=2) as sm_pool, \
                     tc.tile_pool(name="pr", bufs=2) as pr_pool, \
                     tc.tile_pool(name="apl", bufs=2) as apl_pool, \
                     tc.tile_pool(name="ppm", bufs=2, space="PSUM") as ps_main, \
                     tc.tile_pool(name="pph", bufs=1, space="PSUM") as ps_halo, \
                     tc.tile_pool(name="pat3", bufs=1, space="PSUM") as ps_att:

                    qps, kps, vps = {}, {}, {}
                    S_t, Pbf_t, M_t = {}, {}, {}
                    attn_tiles = [None, None]

                    def proj_emit(br):
                        wsb = wpool.tile([128, NCC, SPLIT], bf16, tag="wsb",
                                         name=f"wsb{br}")
                        for ci in range(NCC):
                            eng = nc.sync if ci % 2 == 0 else nc.scalar
                            eng.dma_start(out=wsb[:, ci, :],
                                          in_=wp[br, ci * 128:(ci + 1) * 128, :])
                        qp = proj_pool.tile([128, NCC, BS, TB], bf16, tag="qp",
                                            name=f"qp{br}")
                        kp = proj_pool.tile([128, NCC, BS, TK], bf16, tag="kp",
                                            name=f"kp{br}")
                        vp = proj_pool.tile([128, NCC, BS, TK], bf16, tag="vp",
                                            name=f"vp{br}", bufs=2)
                        qps[br], kps[br], vps[br] = qp, kp, vp
                        for which, xdram, dst in (("q", xq, qp), ("k", xk, kp), ("v", xv, vp)):
                            ncols = TB if which == "q" else TK
                            xts = []
                            for ci in range(NCC):
                                c0 = br * SPLIT + ci * 128
                                xt = xt_pool.tile([128, BS, TK], bf16, tag="xt",
                                                  name=f"xt{which}{br}_{ci}")[:, :, :ncols]
                                for b in range(BS):
                                    nc.sync.dma_start_transpose(
                                        out=xt[:, b, :], in_=xdram[b, :, c0:c0 + 128])
                                xts.append(xt)
                            for co in range(NCC):
                                ps = ps_main.tile([128, BS, TB], f32, tag="ppm",
                                                  name=f"ps{which}{br}_{co}")
                                if which != "q":
                                    ph = ps_halo.tile([128, BS, 2, HALO], f32, tag="pph",
                                                      name=f"ph{which}{br}_{co}")
                                for ci in range(NCC):
                                    lhsT = wsb[:, ci, co * 128:(co + 1) * 128]
                                    st = dict(start=(ci == 0), stop=(ci == NCC - 1))
                                    for b in range(BS):
                                        if which == "q":
                                            nc.tensor.matmul(ps[:, b, :], lhsT,
                                                             xts[ci][:, b, :], **st)
                                        else:
                                            nc.tensor.matmul(
                                                ps[:, b, :], lhsT,
                                                xts[ci][:, b, HALO:HALO + TB], **st)
                                    if which != "q":
                                        hsrc = xts[ci][:, 0, :]
                                        halo_ap = bass.AP(
                                            tensor=hsrc.tensor, offset=hsrc.offset,
                                            ap=[hsrc.ap[0], [TK, BS],
                                                [HALO + TB, 2], [1, HALO]])
                                        nc.tensor.matmul(ph, lhsT, halo_ap, **st)
                                if which == "q":
                                    bias_arg = biasq_t[:, br, co:co + 1] if has_bias else 0.0
                                    for b in range(BS):
                                        nc.scalar.activation(
                                            out=dst[:, co, b, :], in_=ps[:, b, :],
                                            func=Act.Copy, scale=SCALE, bias=bias_arg)
                                else:
                                    for b in range(BS):
                                        pieces = (
                                            (dst[:, co, b, HALO:HALO + TB], ps[:, b, :],
                                             (HALO, HALO + TB)),
                                            (dst[:, co, b, 0:HALO], ph[:, b, 0, :],
                                             (0, HALO)),
                                            (dst[:, co, b, HALO + TB:TK], ph[:, b, 1, :],
                                             (HALO + TB, TK)),
                                        )
                                        for o_, i_, (m0, m1) in pieces:
                                            if has_bias:
                                                nc.vector.scalar_tensor_tensor(
                                                    out=o_, in0=i_,
                                                    scalar=bias_t[:, br, co:co + 1],
                                                    in1=mask_t[:, b, m0:m1],
                                                    op0=Alu.add, op1=Alu.mult)
                                            else:
                                                nc.scalar.activation(out=o_, in_=i_,
                                                                     func=Act.Copy)

                    def scores_emit(br):
                        dil = DILS[br]
                        qp, kp = qps[br], kps[br]
                        for b in range(BS):
                            S = sm_pool.tile([NG, KS, TB], f32, tag="S",
                                             name=f"S{br}{b}")
                            S_t[(br, b)] = S
                            for j0, nj in ((0, 3), (3, 2)):
                                scp = ps_att.tile([NG, nj, TB], f32, tag="att3",
                                                  name=f"scp{br}{b}{j0}")
                                for cc in range(NCC):
                                    qsl = qp[:, cc, b, :]
                                    qb = bass.AP(tensor=qsl.tensor, offset=qsl.offset,
                                                 ap=[qsl.ap[0], [0, nj], [1, TB]])
                                    ksl = kp[:, cc, b, :]
                                    ksh = bass.AP(
                                        tensor=ksl.tensor,
                                        offset=ksl.offset + (HALO - 2 * dil) + dil * j0,
                                        ap=[ksl.ap[0], [dil, nj], [1, TB]])
                                    pr = pr_pool.tile([128, 3, TB], bf16, tag="pr",
                                                      name=f"pr{br}{b}{j0}_{cc}")
                                    nc.vector.tensor_mul(pr[:, :nj, :], qb, ksh)
                                    for jj in range(nj):
                                        nc.tensor.matmul(
                                            scp[:, jj, :], psum_pat, pr[:, jj, :],
                                            start=(cc == 0), stop=(cc == NCC - 1))
                                nc.scalar.activation(out=S[:, j0:j0 + nj, :], in_=scp,
                                                     func=Act.Copy)
                            if dbg:
                                nc.sync.dma_start(out=dS[br, b], in_=S)

                    def softmax_emit(br):
                        for b in range(BS):
                            S = S_t[(br, b)]
                            nc.scalar.activation(out=S, in_=S, func=Act.Exp)
                            Z = sm_pool.tile([NG, TB], f32, tag="Z", name=f"Z{br}{b}")
                            nc.vector.tensor_reduce(
                                out=Z, in_=S.rearrange("p j t -> p t j"),
                                axis=Ax.X, op=Alu.add)
                            nc.vector.reciprocal(out=Z, in_=Z)
                            nc.vector.tensor_mul(S, S, bcast_mid(Z, KS))
                            Sout = sm_pool.tile([NG, TB, KS], f32, tag="Sout", bufs=1,
                                                name=f"So{br}{b}")
                            nc.vector.tensor_copy(Sout, S.rearrange("p j t -> p t j"))
                            nc.sync.dma_start(
                                out=p_out[b, br * NG:(br + 1) * NG, :, :], in_=Sout)
                            Pbf = sm_pool.tile([NG, KS, TB], bf16, tag="Pbf", bufs=2,
                                               name=f"Pb{br}{b}")
                            nc.vector.tensor_copy(Pbf, S)
                            Pbf_t[(br, b)] = Pbf

                    def repl_emit(br):
                        for b in range(BS):
                            M = sm_pool.tile([128, KS, TB], bf16, tag="M", bufs=2,
                                             name=f"M{br}{b}")
                            M_t[(br, b)] = M
                            for j0, nj in ((0, 3), (3, 2)):
                                mp = ps_att.tile([128, nj, TB], f32, tag="att3",
                                                 name=f"mp{br}{b}{j0}")
                                for jj in range(nj):
                                    nc.tensor.matmul(mp[:, jj, :], erep_pat,
                                                     Pbf_t[(br, b)][:, j0 + jj, :])
                                nc.scalar.activation(out=M[:, j0:j0 + nj, :], in_=mp,
                                                     func=Act.Copy)

                    def apply_emit(br):
                        dil = DILS[br]
                        vp = vps[br]
                        attn_t = attn_pool.tile([128, NCC, BS, TB], bf16, tag="attn",
                                                name=f"attn{br}")
                        attn_tiles[br] = attn_t
                        for b in range(BS):
                            M = M_t[(br, b)]
                            for cc in range(NCC):
                                vsl = vp[:, cc, b, :]
                                vsh = bass.AP(tensor=vsl.tensor,
                                              offset=vsl.offset + (HALO - 2 * dil),
                                              ap=[vsl.ap[0], [dil, KS], [1, TB]])
                                pv = apl_pool.tile([128, KS, TB], bf16, tag="pv", bufs=1,
                                                   name=f"pv{br}{b}{cc}")
                                nc.vector.tensor_mul(pv, vsh, M)
                                t01 = apl_pool.tile([128, TB], bf16, tag="t01", bufs=1,
                                                    name=f"t01_{br}{b}{cc}")
                                nc.vector.tensor_add(t01, pv[:, 0, :], pv[:, 1, :])
                                t23 = apl_pool.tile([128, TB], bf16, tag="t23", bufs=1,
                                                    name=f"t23_{br}{b}{cc}")
                                nc.vector.tensor_add(t23, pv[:, 2, :], pv[:, 3, :])
                                t4 = apl_pool.tile([128, TB], bf16, tag="t4", bufs=1,
                                                   name=f"t4_{br}{b}{cc}")
                                nc.vector.tensor_add(t4, t01, t23)
                                nc.vector.tensor_add(attn_t[:, cc, b, :], t4, pv[:, 4, :])

                    # emission order chosen so no engine's in-order stream
                    # blocks another branch's independent work
                    proj_emit(0)
                    scores_emit(0)
                    proj_emit(1)
                    softmax_emit(0)
                    repl_emit(0)
                    scores_emit(1)
                    softmax_emit(1)
                    repl_emit(1)
                    apply_emit(0)
                    apply_emit(1)

                if dbg:
                    for _br in range(2):
                        nc.sync.dma_start(out=dA[_br], in_=attn_tiles[_br])

                # ===== phase 3: out_proj (split per branch) + residual + LN =====
                with tc.tile_pool(name="wo", bufs=6) as wo_pool, \
                     tc.tile_pool(name="acc", bufs=1) as acc_pool, \
                     tc.tile_pool(name="yln", bufs=2) as y_pool, \
                     tc.tile_pool(name="yst", bufs=4) as st_pool, \
                     tc.tile_pool(name="yp", bufs=8, space="PSUM") as y_ps:
                    NOC = DM // 512
                    NTT = TB // 128
                    accs = {}

                    def outproj_half_a():
                        for b in range(BS):
                            for tt in range(NTT):
                                acc = acc_pool.tile([128, DM], bf16, tag="acc",
                                                    bufs=2 * NTT, name=f"acc{b}{tt}")
                                accs[(b, tt)] = acc
                        for b in range(BS):
                            for oc in range(NOC):
                                osl = slice(oc * 512, (oc + 1) * 512)
                                apss = {}
                                for tt in range(NTT):
                                    apss[tt] = y_ps.tile([128, 512], f32, tag="yy",
                                                         name=f"yA{b}{tt}{oc}", bufs=8)
                                for cc in range(NCC):
                                    wt = wo_pool.tile([128, 512], bf16, tag="wt",
                                                      name=f"wA{b}{oc}{cc}")
                                    eng = nc.sync if cc % 2 == 0 else nc.scalar
                                    eng.dma_start(
                                        out=wt, in_=wot[cc * 128:(cc + 1) * 128, osl])
                                    for tt in range(NTT):
                                        tsl = slice(tt * 128, (tt + 1) * 128)
                                        nc.tensor.matmul(
                                            apss[tt], attn_tiles[0][:, cc, b, tsl], wt,
                                            start=(cc == 0), stop=(cc == NCC - 1))
                                for tt in range(NTT):
                                    nc.scalar.activation(out=accs[(b, tt)][:, osl],
                                                         in_=apss[tt], func=Act.Copy)

                    def outproj_half_b():
                        ysbs = {}
                        for b in range(BS):
                            for tt in range(NTT):
                                ysbs[(b, tt)] = y_pool.tile(
                                    [128, DM], f32, tag="ysb", bufs=2 * NTT,
                                    name=f"ysb{b}{tt}")
                        for oc in range(NOC):
                            osl = slice(oc * 512, (oc + 1) * 512)
                            bpss = {}
                            for b in range(BS):
                                for tt in range(NTT):
                                    bpss[(b, tt)] = y_ps.tile(
                                        [128, 512], f32, tag="yy",
                                        name=f"yB{b}{tt}{oc}", bufs=8)
                            for cc in range(NCC):
                                wt = wo_pool.tile([128, 512], bf16, tag="wt",
                                                  name=f"wB{oc}{cc}")
                                eng = nc.sync if cc % 2 == 0 else nc.scalar
                                eng.dma_start(
                                    out=wt,
                                    in_=wot[(NCC + cc) * 128:(NCC + cc + 1) * 128, osl])
                                for b in range(BS):
                                    for tt in range(NTT):
                                        tsl = slice(tt * 128, (tt + 1) * 128)
                                        nc.tensor.matmul(
                                            bpss[(b, tt)],
                                            attn_tiles[1][:, cc, b, tsl], wt,
                                            start=(cc == 0), stop=(cc == NCC - 1))
                            for b in range(BS):
                                for tt in range(NTT):
                                    nc.vector.tensor_add(
                                        ysbs[(b, tt)][:, osl], bpss[(b, tt)],
                                        accs[(b, tt)][:, osl])

                        tiles = [(b, tt) for b in range(BS) for tt in range(NTT)]
                        qrs = {}
                        for b, tt in tiles:
                            qr = y_pool.tile([128, DM], f32, tag="qr", bufs=4,
                                             name=f"qr{b}{tt}")
                            nc.scalar.dma_start(
                                out=qr, in_=qres[b, tt * 128:(tt + 1) * 128, :])
                            qrs[(b, tt)] = qr
                        for b, tt in tiles:
                            nc.vector.tensor_add(ysbs[(b, tt)], ysbs[(b, tt)],
                                                 qrs[(b, tt)])
                        mvs = {}
                        for b, tt in tiles:
                            ysb = ysbs[(b, tt)]
                            stats = st_pool.tile([128, 4, 6], f32, tag="stats",
                                                 bufs=8, name=f"st{b}{tt}")
                            for sgi in range(4):
                                nc.vector.bn_stats(
                                    out=stats[:, sgi, :],
                                    in_=ysb[:, sgi * 512:(sgi + 1) * 512])
                            mv = st_pool.tile([128, 2], f32, tag="mv", bufs=8,
                                              name=f"mv{b}{tt}")
                            nc.vector.bn_aggr(out=mv, in_=stats)
                            mvs[(b, tt)] = mv
                        sds = {}
                        for b, tt in tiles:
                            sd = st_pool.tile([128, 1], f32, tag="sd", bufs=8,
                                              name=f"sd{b}{tt}")
                            nc.scalar.activation(out=sd, in_=mvs[(b, tt)][:, 1:2],
                                                 func=Act.Sqrt, bias=eps_t)
                            nc.vector.reciprocal(out=sd, in_=sd)
                            sds[(b, tt)] = sd
                        for b, tt in tiles:
                            yn = y_pool.tile([128, DM], f32, tag="yn", bufs=2,
                                             name=f"yn{b}{tt}")
                            nc.vector.scalar_tensor_tensor(
                                out=yn, in0=ysbs[(b, tt)], scalar=mvs[(b, tt)][:, 0:1],
                                in1=bcast_free(sds[(b, tt)], DM), op0=Alu.subtract,
                                op1=Alu.mult)
                            if has_affine:
                                nc.vector.tensor_mul(yn, yn, lnw_t)
                                nc.vector.tensor_add(yn, yn, lnb_t)
                            nc.sync.dma_start(
                                out=y_out[b, tt * 128:(tt + 1) * 128, :], in_=yn)

                    outproj_half_a()
                    outproj_half_b()

    nc.compile()
    return nc


def _get_program(has_bias, has_affine):
    key = (has_bias, has_affine)
    if key not in _CACHE:
        _CACHE[key] = _build_program(has_bias, has_affine)
    return _CACHE[key]


def _prep_inputs(query, key, values, W0, b0, W1, b1, Wo, bo, ln_w, ln_b):
    perm_o, perm_a = _perms()
    wp = np.stack([
        np.ascontiguousarray(W0[perm_o, :].T),
        np.ascontiguousarray(W1[perm_o, :].T),
    ]).astype(BF16)
    row_perm = np.concatenate([perm_a, SPLIT + perm_a])
    wot = np.ascontiguousarray(Wo.T[row_perm, :]).astype(BF16)

    p = np.arange(128)
    pat_sum = (p[:, None] % NG == np.arange(NG)[None, :]).astype(BF16)
    pat_rep = (np.arange(NG)[:, None] == p[None, :] % NG).astype(BF16)

    has_bias = bool(np.any(b0) or np.any(b1))
    has_affine = bool(np.any(ln_b) or not np.all(ln_w == 1.0))

    shared = dict(wp=wp, wot=wot, pat_sum=pat_sum, pat_rep=pat_rep)
    if has_bias:
        bp = np.stack([b0[perm_o], b1[perm_o]])            # [br, c_new]
        bt = np.ascontiguousarray(
            bp.reshape(2, NCC, 128).transpose(2, 0, 1)).astype(np.float32)
        shared["bias"] = bt
        shared["bias_q"] = (bt * SCALE).astype(np.float32)
    if has_affine:
        shared["ln_w"] = np.asarray(ln_w, np.float32)
        shared["ln_b"] = np.asarray(ln_b, np.float32)

    qres_full = (query + bo[None, None, :]).astype(np.float32)
    qbf = query.astype(BF16)
    kbf = key.astype(BF16)
    vbf = values.astype(BF16)

    in_maps = []
    for c in range(NCORES):
        t0 = c * TB
        lo, hi = t0 - HALO, t0 + TB + HALO
        clo, chi = max(lo, 0), min(hi, SEQ)
        xk = np.zeros((BS, TK, DM), BF16)
        xv = np.zeros((BS, TK, DM), BF16)
        xk[:, clo - lo:clo - lo + (chi - clo)] = kbf[:, clo:chi]
        xv[:, clo - lo:clo - lo + (chi - clo)] = vbf[:, clo:chi]
        m = dict(
            xq=np.ascontiguousarray(qbf[:, t0:t0 + TB]),
            xk=xk, xv=xv,
            qres=np.ascontiguousarray(qres_full[:, t0:t0 + TB]),
            **shared,
        )
        if has_bias:
            mk = np.zeros((BS, TK), np.float32)
            mk[:, clo - lo:clo - lo + (chi - clo)] = 1.0
            m["maskkv"] = mk
        in_maps.append(m)
    return in_maps, has_bias, has_affine


def _run(inputs, trace=False):
    from concourse.bass_utils import run_bass_kernel_spmd

    in_maps, has_bias, has_affine = _prep_inputs(
        inputs["query"], inputs["key"], inputs["values"],
        inputs["W0"], inputs["b0"], inputs["W1"], inputs["b1"],
        inputs["Wo"], inputs["bo"], inputs["ln_w"], inputs["ln_b"])
    nc = _get_program(has_bias, has_affine)
    res = run_bass_kernel_spmd(nc, in_maps, list(range(NCORES)), trace=trace)
    y = np.empty((BS, SEQ, DM), np.float32)
    probs = np.empty((BS, 2 * NG, SEQ, KS), np.float32)
    for c in range(NCORES):
        y[:, c * TB:(c + 1) * TB, :] = res.results[c]["y"]
        probs[:, :, c * TB:(c + 1) * TB, :] = res.results[c]["probs"]
    return (y, probs), res


def kernel(**inputs):
    inputs = {k: np.asarray(v) for k, v in inputs.items()}
    (y, probs), _ = _run(inputs, trace=False)
    return y, probs
